# revision 13
# baseline (speedup 1.0000x reference)
"""Trainium2 Bass kernel for nn_EnhancedSubtractionUnit.

B=8, C=256, H=W=64. Data-parallel over batch: 1 sample per NeuronCore (8 cores).

Per-core pipeline (channel-major layout [C_part, H, W], C split into 2 blocks
of 128 partitions; spatial padded to 66x66 for SAME 3x3 convs):
  1. conv1 (cat(x_low,x_high) 512->256, BN folded) as bf16 hi/lo 3-term
     matmuls (offset path needs ~16 mantissa bits); inputs shipped pre-padded
     and pre-split from host so phase 0 is pure DMA; ReLU h written via ACT,
     then split to bf16 hi/lo pair on DVE
  2. conv2 tap-packed: O[(c,t), p] = W2 @ h (6 matmuls/tile instead of 54),
     O split to bf16 hi/lo padded tiles, then the 9 shifted tap reductions run
     on the PE with an exact 0/1 selection lhsT (2 terms), offsets exact to
     ~2^-17
  3. offsets transposed to pixel-partition layout (PE transpose), grid math on
     DVE (exact floor via int-cast + is_gt correction), bilinear weights and
     gather indices
  4. grid_sample: indirect-DMA row gather from a host-prepared transposed
     table with guard rows (adjacent-pixel pairs -> 2 gathers / 4 taps),
     bilinear combine via scalar_tensor_tensor FMA, PE transpose back to
     channel-major, diff = x_low - aligned (x_low read from resident padded
     SBUF tile, no re-DMA)
  5. SE scales computed ANALYTICALLY before any dw conv: spatial mean of
     conv_k(diff) = W_k . (9 shifted-window sums of diff), computed from 9
     subrectangle sums (inclusion-exclusion of border strips/corners) on DVE
     + 108 tiny matvecs on PE; then sigmoid -> s_k; the three branch kernels
     are merged on DVE into ONE 256->256 conv: W_m = sum_k s_k (x) dw_k
     (per-output-channel scale broadcast via a rank-1 PE matmul), so phase 5
     runs one f32r conv instead of three
  6. attention conv tap-packed in f32r: A[(ci,t), p] (2 matmuls/tile) +
     9 shifted 0/1-selection matmuls that also broadcast the 1-channel result
     to all 128 partitions; sigmoid; out = attn*diff + x_low
"""
import os
import sys

sys.path.insert(0, "/opt/trn_rl_repo")

import numpy as np
import concourse.bass as bass
import concourse.bacc as bacc
import concourse.tile as tile
from concourse import mybir
from concourse.bass_utils import run_bass_kernel_spmd

F32 = mybir.dt.float32
F32R = mybir.dt.float32r
BF16 = mybir.dt.bfloat16
I32 = mybir.dt.int32
ALU = mybir.AluOpType
ACT = mybir.ActivationFunctionType

B, C, H, W = 8, 256, 64, 64
HW = H * W
PH, PW = H + 2, W + 2  # padded spatial
NCORES = 8
EPS = 1e-5
TBL_ROWS = 4360  # >= 66*66 guard-padded 2x2-patch table rows

_nc_cache = {}


def _border_zero(nc, t, np_, dt=None):
    """Zero the 1-px border of a padded [np_, PH, PW] tile."""
    v = t[:].bitcast(F32) if dt is None else t[:]
    nc.gpsimd.memset(v[:, 0:1, :], 0.0)
    nc.gpsimd.memset(v[:, PH - 1:PH, :], 0.0)
    nc.vector.memset(v[:, 1:PH - 1, 0:1], 0.0)
    nc.vector.memset(v[:, 1:PH - 1, PW - 1:PW], 0.0)


def _emit_body(nc, tc, prm):
    """Emit one full forward pass. prm: dict of DRAM param handles."""
    phases = set(os.environ.get(
        "KERNEL_PHASES", "conv1,conv2,gather,dw,attn").split(","))
    ctx_pools = []

    def pool(name, bufs=1, space="SBUF"):
        p = tc.alloc_tile_pool(name=name, bufs=bufs, space=space)
        ctx_pools.append(p)
        return p

    pc = pool("const", 1)
    ppsum = pool("ppsum", 2, space="PSUM")
    ptpsum = pool("ptpsum", 1, space="PSUM")
    psmall = pool("psmall", 2, space="PSUM")
    phoff = tc.alloc_tile_pool(name="phoff", bufs=1)     # h_pad, dies after ph3
    poff = tc.alloc_tile_pool(name="poff", bufs=1)       # off/O, dies after ph3
    pconv1 = tc.alloc_tile_pool(name="pconv1", bufs=1)   # x splits/w1, dies after conv1

    # ---------------- constant loads ----------------
    w2ph_sb = pc.tile([128, 36], BF16, name="w2ph_sb")
    nc.sync.dma_start(w2ph_sb[:], prm["w2ph"][:])
    w2pl_sb = pc.tile([128, 36], BF16, name="w2pl_sb")
    nc.sync.dma_start(w2pl_sb[:], prm["w2pl"][:])
    sel_sb = pc.tile([18, 18], BF16, name="sel_sb")
    nc.sync.dma_start(sel_sb[:], prm["sel"][:])
    asel_sb = pc.tile([41, 1152], F32R, name="asel_sb")
    nc.sync.dma_start(asel_sb[:], prm["asel"][:])
    saw2_sb = pc.tile([128, 18], F32R, name="saw2_sb")
    nc.sync.dma_start(saw2_sb[:], prm["sawT2"][:])
    ones_sb = pc.tile([1, 128], F32R, name="ones_sb")
    nc.sync.dma_start(ones_sb[:], prm["ones1"][:])
    bxy_sb = pc.tile([128, 64], F32, name="bxy_sb")
    nc.sync.dma_start(bxy_sb[:], prm["bxy"][:])
    id_sb = pc.tile([128, 128], F32, name="id_sb")
    nc.sync.dma_start(id_sb[:], prm["ident"][:])
    b1_sb = pc.tile([128, 2], F32, name="b1_sb")
    nc.sync.dma_start(b1_sb[:], prm["b1"][:])
    sab_sb = pc.tile([128, 1], F32, name="sab_sb")
    nc.sync.dma_start(sab_sb[:], prm["sab_bc"][:])
    db_sb = pc.tile([128, 6], F32, name="db_sb")
    nc.sync.dma_start(db_sb[:], prm["db2"][:])
    dbhw_sb = pc.tile([128, 6], F32, name="dbhw_sb")
    nc.sync.dma_start(dbhw_sb[:], prm["db_hw"][:])
    se1_sb = pc.tile([128, 384], F32, name="se1_sb")
    nc.sync.dma_start(se1_sb[:], prm["se1T"][:])
    se2_sb = pc.tile([64, 768], F32, name="se2_sb")
    nc.sync.dma_start(se2_sb[:], prm["se2T"][:])
    se1b_sb = pc.tile([64, 3], F32, name="se1b_sb")
    nc.sync.dma_start(se1b_sb[:], prm["se1b2"][:])
    se2b_sb = pc.tile([128, 6], F32, name="se2b_sb")
    nc.sync.dma_start(se2b_sb[:], prm["se2b2"][:])

    # ---------------- phase 0: load pre-padded pre-split conv inputs --------
    xcat_hi = []
    xcat_lo = []
    for pname in ["xlph", "xlpl", "xhph", "xhpl"]:
        for cio in range(2):
            t = pconv1.tile([128, PH, PW], BF16, name=f"{pname}{cio}")
            nc.sync.dma_start(t[:], prm[pname][cio * 128:(cio + 1) * 128, :, :])
            (xcat_hi if pname.endswith("h") else xcat_lo).append(t)
    # order: xcat_hi = [xl0, xl1, xh0, xh1], xcat_lo likewise

    h_hi = []
    h_lo = []
    for co in range(2):
        t = phoff.tile([128, PH, PW], BF16, name=f"h_hi{co}")
        _border_zero(nc, t, 128, BF16)
        h_hi.append(t)
        t = phoff.tile([128, PH, PW], BF16, name=f"h_lo{co}")
        _border_zero(nc, t, 128, BF16)
        h_lo.append(t)

    # ------------ phase 1+2: conv1 (bf16 hi/lo 3-term) interleaved with
    # tap-packed conv2 so the PE never drains between them --------
    w1h = []
    w1l = []
    for co in range(2 if "conv1" in phases else 0):
        t = pconv1.tile([128, 4608], BF16, name=f"w1h{co}")
        nc.sync.dma_start(t[:], prm["w1Th"][:, co * 4608:(co + 1) * 4608])
        w1h.append(t)
        t = pconv1.tile([128, 4608], BF16, name=f"w1l{co}")
        nc.sync.dma_start(t[:], prm["w1Tl"][:, co * 4608:(co + 1) * 4608])
        w1l.append(t)

    off_sb = poff.tile([2, HW], F32, name="off_sb")
    do_c2 = "conv2" in phases
    if do_c2:
        O_hi = poff.tile([18, PH, PW], BF16, name="O_hi")
        _border_zero(nc, O_hi, 18, BF16)
        O_lo = poff.tile([18, PH, PW], BF16, name="O_lo")
        _border_zero(nc, O_lo, 18, BF16)
    else:
        nc.gpsimd.memset(off_sb[:], 0.0)

    def conv2_O(nt):
        pO = psmall.tile([18, 512], F32, name="c2psum", tag="small18")
        for cib in range(2):
            lh = w2ph_sb[:, cib * 18:(cib + 1) * 18]
            ll = w2pl_sb[:, cib * 18:(cib + 1) * 18]
            rh = h_hi[cib][:, nt * 8 + 1:nt * 8 + 9, 1:65]
            rl = h_lo[cib][:, nt * 8 + 1:nt * 8 + 9, 1:65]
            nc.tensor.matmul(pO[:], lh, rh, start=(cib == 0), stop=False)
            nc.tensor.matmul(pO[:], lh, rl, start=False, stop=False)
            nc.tensor.matmul(pO[:], ll, rh, start=False, stop=(cib == 1))
        ohv = O_hi[:, nt * 8 + 1:nt * 8 + 9, 1:65]
        nc.scalar.activation(ohv, pO[:], ACT.Identity, scale=1.0)
        nc.vector.tensor_sub(O_lo[:, nt * 8 + 1:nt * 8 + 9, 1:65], pO[:], ohv)

    def conv2_reduce(nt):
        pr_t = psmall.tile([18, 512], F32, name="rpsum", tag="small18")
        pr = pr_t[0:2, :]
        first = True
        for t9 in range(9):
            dy, dx = t9 // 3 - 1, t9 % 3 - 1
            sl = sel_sb[:, t9 * 2:t9 * 2 + 2]
            wh = O_hi[:, nt * 8 + 1 + dy:nt * 8 + 9 + dy, 1 + dx:65 + dx]
            wl = O_lo[:, nt * 8 + 1 + dy:nt * 8 + 9 + dy, 1 + dx:65 + dx]
            nc.tensor.matmul(pr, sl, wh, start=first, stop=False)
            nc.tensor.matmul(pr, sl, wl, start=False, stop=(t9 == 8))
            first = False
        nc.vector.tensor_copy(off_sb[:, nt * 512:(nt + 1) * 512], pr)

    for nt in range(8 if "conv1" in phases else 0):
        for co in range(2):
            ps = ppsum.tile([128, 512], F32, name="c1psum", tag="c1psum")
            first = True
            for t9 in range(9):
                dy, dx = t9 // 3 - 1, t9 % 3 - 1
                for ci in range(4):
                    col = (t9 * 4 + ci) * 128
                    rhs_hi = xcat_hi[ci][:, nt * 8 + 1 + dy:nt * 8 + 9 + dy,
                                         1 + dx:65 + dx]
                    rhs_lo = xcat_lo[ci][:, nt * 8 + 1 + dy:nt * 8 + 9 + dy,
                                         1 + dx:65 + dx]
                    last = (t9 == 8 and ci == 3)
                    nc.tensor.matmul(ps[:], w1h[co][:, col:col + 128], rhs_hi,
                                     start=first, stop=False)
                    nc.tensor.matmul(ps[:], w1h[co][:, col:col + 128], rhs_lo,
                                     start=False, stop=False)
                    nc.tensor.matmul(ps[:], w1l[co][:, col:col + 128], rhs_hi,
                                     start=False, stop=last)
                    first = False
            hstg = pconv1.tile([128, 512], F32, name="hstg", tag="hstg", bufs=2)
            nc.scalar.activation(hstg[:], ps[:], ACT.Relu,
                                 bias=b1_sb[:, co:co + 1], scale=1.0)
            hiv = h_hi[co][:, nt * 8 + 1:nt * 8 + 9, 1:65]
            nc.vector.tensor_copy(hiv, hstg[:])
            nc.vector.tensor_sub(h_lo[co][:, nt * 8 + 1:nt * 8 + 9, 1:65],
                                 hstg[:], hiv)
        if do_c2:
            conv2_O(nt)
            if nt >= 2:
                conv2_reduce(nt - 2)
    if do_c2 and "conv1" in phases:
        conv2_reduce(6)
        conv2_reduce(7)

    pconv1.release()

    # ---------------- phase 3: transpose offsets + grid math ----------------
    # pixel-partition layout: pixel p = j*128 + i -> tile[i, j], j in [0,32)
    pst_t = ptpsum.tile([128, 128], F32, name="offT_psum", tag="tp", bufs=1)
    pst = pst_t[:, 0:64]
    for j in range(32):
        nc.tensor.transpose(pst_t[:, 2 * j:2 * j + 2], off_sb[:, j * 128:(j + 1) * 128],
                            id_sb[:2, :2])
    ixiy = pc.tile([128, 64], F32, name="ixiy")
    # ix/iy = 32*offset + base (scale folded into w2 on host; bxy holds base)
    nc.vector.tensor_add(ixiy[:], pst, bxy_sb[:])
    poff.release()
    phoff.release()
    ix = ixiy[:, 0::2]
    iy = ixiy[:, 1::2]

    G = [128, 32]

    def f32t(name):
        return pc.tile(G, F32, name=name)

    # exact floor via int cast + correction
    xi_i = pc.tile(G, I32, name="xi_i")
    nc.vector.tensor_copy(xi_i[:], ix)
    fx0 = f32t("fx0")
    nc.vector.tensor_copy(fx0[:], xi_i[:])
    corr = f32t("corr")
    nc.vector.tensor_tensor(corr[:], fx0[:], ix, op=ALU.is_gt)
    nc.vector.tensor_sub(fx0[:], fx0[:], corr[:])
    yi_i = pc.tile(G, I32, name="yi_i")
    nc.vector.tensor_copy(yi_i[:], iy)
    fy0 = f32t("fy0")
    nc.vector.tensor_copy(fy0[:], yi_i[:])
    corr2 = f32t("corr2")
    nc.vector.tensor_tensor(corr2[:], fy0[:], iy, op=ALU.is_gt)
    nc.vector.tensor_sub(fy0[:], fy0[:], corr2[:])

    wx = f32t("wx")
    nc.vector.tensor_sub(wx[:], ix, fx0[:])
    wy = f32t("wy")
    nc.vector.tensor_sub(wy[:], iy, fy0[:])

    def valid01(src, name):
        v0a = f32t(name + "_0a")
        nc.vector.tensor_scalar(v0a[:], src[:], 0.0, None, op0=ALU.is_ge)
        v0b = f32t(name + "_0b")
        nc.vector.tensor_scalar(v0b[:], src[:], 63.0, None, op0=ALU.is_le)
        v0 = f32t(name + "_0")
        nc.vector.tensor_mul(v0[:], v0a[:], v0b[:])
        v1a = f32t(name + "_1a")
        nc.vector.tensor_scalar(v1a[:], src[:], -1.0, None, op0=ALU.is_ge)
        v1b = f32t(name + "_1b")
        nc.vector.tensor_scalar(v1b[:], src[:], 62.0, None, op0=ALU.is_le)
        v1 = f32t(name + "_1")
        nc.vector.tensor_mul(v1[:], v1a[:], v1b[:])
        return v0, v1

    vx0, vx1 = valid01(fx0, "vx")
    vy0, vy1 = valid01(fy0, "vy")

    # clamped addresses (+1 guard-row shift folded into xc1)
    xc1 = f32t("xc1")  # clamp(fx0, -1, 64) + 1 == clamp(fx0+1, 0, 65)
    nc.vector.tensor_scalar(xc1[:], fx0[:], -1.0, 64.0, op0=ALU.max, op1=ALU.min)
    nc.vector.tensor_scalar_add(xc1[:], xc1[:], 1.0)
    yc1g = f32t("yc1g")  # clamp(fy0, -1, 64) + 1
    nc.vector.tensor_scalar(yc1g[:], fy0[:], -1.0, 64.0, op0=ALU.max, op1=ALU.min)
    nc.vector.tensor_scalar_add(yc1g[:], yc1g[:], 1.0)
    idx_f = f32t("idx_f")
    nc.vector.scalar_tensor_tensor(idx_f[:], yc1g[:], 66.0, xc1[:],
                                   op0=ALU.mult, op1=ALU.add)
    idxP = pc.tile(G, I32, name="idxP")
    nc.vector.tensor_copy(idxP[:], idx_f[:])

    # bilinear weights, validity folded in
    u = f32t("u")  # (1-wx)*vx0
    nc.vector.tensor_scalar(u[:], wx[:], -1.0, 1.0, op0=ALU.mult, op1=ALU.add)
    nc.vector.tensor_mul(u[:], u[:], vx0[:])
    v = f32t("v")  # (1-wy)*vy0
    nc.vector.tensor_scalar(v[:], wy[:], -1.0, 1.0, op0=ALU.mult, op1=ALU.add)
    nc.vector.tensor_mul(v[:], v[:], vy0[:])
    wxv = f32t("wxv")
    nc.vector.tensor_mul(wxv[:], wx[:], vx1[:])
    wyv = f32t("wyv")
    nc.vector.tensor_mul(wyv[:], wy[:], vy1[:])
    w00 = f32t("w00")
    nc.vector.tensor_mul(w00[:], u[:], v[:])
    w01 = f32t("w01")
    nc.vector.tensor_mul(w01[:], wxv[:], v[:])
    w10 = f32t("w10")
    nc.vector.tensor_mul(w10[:], u[:], wyv[:])
    w11 = f32t("w11")
    nc.vector.tensor_mul(w11[:], wxv[:], wyv[:])

    # ---------------- phase 4: gather + bilinear + diff ----------------
    pdiff = tc.alloc_tile_pool(name="pdiff", bufs=1)
    ctx_pools.append(pdiff)
    pwork = tc.alloc_tile_pool(name="pwork", bufs=2)
    diff_pad = []
    for co in range(2):
        t = pdiff.tile([128, PH, PW], F32R, name=f"diff_pad{co}")
        _border_zero(nc, t, 128)
        diff_pad.append(t)

    for j in range(32):
        gP = pwork.tile([128, 1024], BF16, name="gP", tag="gP")
        if "gather" in phases:
            nc.gpsimd.indirect_dma_start(
                out=gP[:], out_offset=None, in_=prm["xT2"][:],
                in_offset=bass.IndirectOffsetOnAxis(ap=idxP[:, j:j + 1], axis=0))
        else:
            nc.sync.dma_start(gP[:], prm["xT2"][j * 64:j * 64 + 128, :])
        acc = pwork.tile([128, 256], F32, name="acc", tag="acc")
        nc.vector.tensor_scalar_mul(acc[:], gP[:, 0:256], w00[:, j:j + 1])
        nc.vector.scalar_tensor_tensor(acc[:], gP[:, 256:512], w01[:, j:j + 1],
                                       acc[:], op0=ALU.mult, op1=ALU.add)
        nc.vector.scalar_tensor_tensor(acc[:], gP[:, 512:768], w10[:, j:j + 1],
                                       acc[:], op0=ALU.mult, op1=ALU.add)
        nc.vector.scalar_tensor_tensor(acc[:], gP[:, 768:1024], w11[:, j:j + 1],
                                       acc[:], op0=ALU.mult, op1=ALU.add)
        # transpose [128px, 256ch] back to channel-major, diff = x_low - aligned
        for co in range(2):
            pt = ptpsum.tile([128, 128], F32, name="alT_psum", tag="tp", bufs=1)
            nc.tensor.transpose(pt[:], acc[:, co * 128:(co + 1) * 128], id_sb[:])
            xlw = pwork.tile([128, 2, 64], F32, name="xlw", tag="xlw")
            nc.sync.dma_start(xlw[:], prm["xlp"][co * 128:(co + 1) * 128,
                                                 2 * j + 1:2 * j + 3, 1:65])
            nc.vector.tensor_sub(
                diff_pad[co][:, 2 * j + 1:2 * j + 3, 1:65],
                xlw[:], pt[:])

    # ------- phase 5a: analytic SE pooling + merged dw kernel build -------
    pwork.release()
    pfused = tc.alloc_tile_pool(name="pfused", bufs=1)
    ctx_pools.append(pfused)
    pdwxd = tc.alloc_tile_pool(name="pdwxd", bufs=1)
    fused_pad = []
    for co in range(2):
        t = pfused.tile([128, PH, PW], F32R, name=f"fused_pad{co}")
        _border_zero(nc, t, 128)
        fused_pad.append(t)

    do_dw = "dw" in phases
    if not do_dw:
        for co in range(2):
            nc.gpsimd.memset(fused_pad[co][:].bitcast(F32), 0.0)

    if do_dw:
        dwk_sb = []
        for k in range(3):
            t = pdwxd.tile([128, 4608], F32R, name=f"dwk_sb{k}")
            nc.sync.dma_start(t[:], prm[f"dwT{k}"][:])
            dwk_sb.append(t)

        # 9 shifted-window sums of diff per input channel (inclusion-exclusion)
        m9 = []
        for co in range(2):
            dv = diff_pad[co][:].bitcast(F32)
            Tt = pc.tile([128, 1], F32, name=f"Tsum{co}", tag=f"Tsum{co}")
            nc.vector.reduce_sum(Tt[:], dv[:, 1:65, 1:65],
                                 axis=mybir.AxisListType.XY)
            r0 = pc.tile([128, 1], F32, name=f"r0_{co}", tag=f"r0_{co}")
            nc.vector.reduce_sum(r0[:], dv[:, 1:2, 1:65],
                                 axis=mybir.AxisListType.XY)
            r63 = pc.tile([128, 1], F32, name=f"r63_{co}", tag=f"r63_{co}")
            nc.vector.reduce_sum(r63[:], dv[:, 64:65, 1:65],
                                 axis=mybir.AxisListType.XY)
            c0 = pc.tile([128, 1], F32, name=f"c0_{co}", tag=f"c0_{co}")
            nc.vector.reduce_sum(c0[:], dv[:, 1:65, 1:2],
                                 axis=mybir.AxisListType.XY)
            c63 = pc.tile([128, 1], F32, name=f"c63_{co}", tag=f"c63_{co}")
            nc.vector.reduce_sum(c63[:], dv[:, 1:65, 64:65],
                                 axis=mybir.AxisListType.XY)
            m = pc.tile([128, 18], F32R, name=f"m9_{co}", tag=f"m9_{co}")
            # rows R(dy): dy=-1 -> r63 excluded, dy=+1 -> r0; cols same with dx
            # corners added back when both dy,dx nonzero
            corner = {(-1, -1): dv[:, 64:65, 64:65], (-1, 1): dv[:, 64:65, 1:2],
                      (1, -1): dv[:, 1:2, 64:65], (1, 1): dv[:, 1:2, 1:2]}
            rex = {-1: r63, 1: r0}
            cex = {-1: c63, 1: c0}
            tmp = pc.tile([128, 1], F32, name=f"mtmp{co}", tag=f"mtmp{co}")
            for dy in (-1, 0, 1):
                for dx in (-1, 0, 1):
                    t9 = (dy + 1) * 3 + (dx + 1)
                    # duplicated column pair: f32r matmul dst must be 2 wide
                    for dup in range(2):
                        dst = m[:, 2 * t9 + dup:2 * t9 + dup + 1]
                        if dy == 0 and dx == 0:
                            nc.vector.tensor_copy(dst, Tt[:])
                        elif dy == 0:
                            nc.vector.tensor_sub(dst, Tt[:], cex[dx][:])
                        elif dx == 0:
                            nc.vector.tensor_sub(dst, Tt[:], rex[dy][:])
                        else:
                            nc.vector.tensor_sub(tmp[:], Tt[:], rex[dy][:])
                            nc.vector.tensor_sub(tmp[:], tmp[:], cex[dx][:])
                            nc.vector.tensor_add(dst, tmp[:], corner[(dy, dx)])
            m9.append(m)

        # pooled_k[co*128+m] = sum_{t,ci} dw_k[(t,ci,co) blk] . m9[ci][:, t]
        s_t = [[None, None], [None, None], [None, None]]
        for k in range(3):
            pooled = []
            for co in range(2):
                pp = psmall.tile([128, 2], F32, name="poolps", tag="vec128", bufs=1)
                first = True
                for t9 in range(9):
                    for ci in range(2):
                        col = ((t9 * 2 + ci) * 2 + co) * 128
                        nc.tensor.matmul(pp[:], dwk_sb[k][:, col:col + 128],
                                         m9[ci][:, 2 * t9:2 * t9 + 2],
                                         start=first,
                                         stop=(t9 == 8 and ci == 1))
                        first = False
                p_t = pc.tile([128, 1], F32, name=f"pooled{k}{co}",
                              tag=f"pooled{k}{co}")
                nc.scalar.activation(p_t[:], pp[:, 0:1], ACT.Identity,
                                     bias=dbhw_sb[:, 2 * k + co:2 * k + co + 1],
                                     scale=1.0)
                pooled.append(p_t)
            pse1_t = psmall.tile([128, 2], F32, name="pse1", tag="vec128", bufs=1)
            pse1 = pse1_t[0:64, 0:1]
            nc.tensor.matmul(pse1, se1_sb[:, k * 128:k * 128 + 64],
                             pooled[0][:], start=True, stop=False)
            nc.tensor.matmul(pse1, se1_sb[:, k * 128 + 64:k * 128 + 128],
                             pooled[1][:], start=False, stop=True)
            h1 = pc.tile([64, 1], F32, name="h1", tag="h1")
            nc.scalar.activation(h1[:], pse1, ACT.Relu,
                                 bias=se1b_sb[:, k:k + 1], scale=1.0)
            for co in range(2):
                pse2_t = psmall.tile([128, 2], F32, name="pse2", tag="vec128", bufs=1)
                pse2 = pse2_t[:, 0:1]
                nc.tensor.matmul(
                    pse2, se2_sb[:, (k * 2 + co) * 128:(k * 2 + co + 1) * 128],
                    h1[:], start=True, stop=True)
                st = pc.tile([128, 1], F32, name=f"s{k}{co}", tag=f"s{k}{co}")
                nc.scalar.activation(st[:], pse2, ACT.Sigmoid,
                                     bias=se2b_sb[:, 2 * k + co:2 * k + co + 1],
                                     scale=1.0)
                s_t[k][co] = st

        # transpose the six s vectors into one [1, 768] row (k-major, co, m)
        sT_sb = pc.tile([1, 768], F32R, name="sT_sb")
        for k in range(3):
            for co in range(2):
                prow = psmall.tile([1, 128], F32, name="psT", tag="row1", bufs=1)
                nc.tensor.transpose(prow[:], s_t[k][co][:], id_sb[:])
                nc.vector.tensor_copy(
                    sT_sb[:, k * 256 + co * 128:k * 256 + (co + 1) * 128],
                    prow[:])
        # rank-1 broadcast: sbc[p, k*256+co*128+m] = s_k[co*128+m] for all p
        sbc_sb = pc.tile([128, 768], F32, name="sbc_sb")
        for k in range(3):
            psbc = psmall.tile([128, 256], F32, name="psbc", tag="psbc", bufs=1)
            nc.tensor.matmul(psbc[:], ones_sb[:],
                             sT_sb[:, k * 256:(k + 1) * 256], start=True,
                             stop=True)
            nc.vector.tensor_copy(sbc_sb[:, k * 256:(k + 1) * 256], psbc[:])

        # merged kernel W_m = sum_k s_k * dw_k  (broadcast s over partitions
        # and the 18 (t,ci) column blocks)
        wm_sb = pdwxd.tile([128, 4608], F32R, name="wm_sb")
        wsc_sb = pdwxd.tile([128, 4608], F32R, name="wsc_sb")

        def sbk(k):
            return sbc_sb[:, k * 256:(k + 1) * 256].rearrange(
                "p (o m) -> p o m", o=1).broadcast_to([128, 18, 256])

        wmv = wm_sb[:].rearrange("p (t m) -> p t m", t=18)
        wscv = wsc_sb[:].rearrange("p (t m) -> p t m", t=18)
        dw0 = dwk_sb[0][:].bitcast(F32).rearrange("p (t m) -> p t m", t=18)
        dw1 = dwk_sb[1][:].bitcast(F32).rearrange("p (t m) -> p t m", t=18)
        dw2 = dwk_sb[2][:].bitcast(F32).rearrange("p (t m) -> p t m", t=18)
        nc.vector.tensor_mul(wmv, dw0, sbk(0))
        nc.vector.tensor_mul(wscv, dw1, sbk(1))
        nc.vector.tensor_add(wmv, wmv.bitcast(F32), wscv.bitcast(F32))
        nc.vector.tensor_mul(wscv, dw2, sbk(2))
        nc.vector.tensor_add(wmv, wmv.bitcast(F32), wscv.bitcast(F32))

        # fused bias b_m[c] = sum_k s_k[c] db_k[c]
        bm = []
        for co in range(2):
            bt = pc.tile([128, 1], F32, name=f"bm{co}", tag=f"bm{co}")
            nc.vector.tensor_mul(bt[:], db_sb[:, co:co + 1], s_t[0][co][:])
            # bt = db0*s0 ; then += db1*s1 ; += db2*s2
            nc.vector.scalar_tensor_tensor(
                bt[:], db_sb[:, 2 + co:3 + co], s_t[1][co][:], bt[:],
                op0=ALU.mult, op1=ALU.add)
            nc.vector.scalar_tensor_tensor(
                bt[:], db_sb[:, 4 + co:5 + co], s_t[2][co][:], bt[:],
                op0=ALU.mult, op1=ALU.add)
            bm.append(bt)

        # ---------------- phase 5b: single merged 3x3 conv ----------------
        for co in range(2):
            for nt in range(8):
                ps = ppsum.tile([128, 512], F32, name="dwpsum", tag="c1psum")
                first = True
                for t9 in range(9):
                    dy, dx = t9 // 3 - 1, t9 % 3 - 1
                    for ci in range(2):
                        col = ((t9 * 2 + ci) * 2 + co) * 128
                        nc.tensor.matmul(
                            ps[:],
                            wm_sb[:, col:col + 128],
                            diff_pad[ci][:, nt * 8 + 1 + dy:nt * 8 + 9 + dy,
                                         1 + dx:65 + dx],
                            start=first, stop=(t9 == 8 and ci == 1))
                        first = False
                nc.scalar.activation(
                    fused_pad[co][:, nt * 8 + 1:nt * 8 + 9, 1:65], ps[:],
                    ACT.Identity, bias=bm[co][:, 0:1], scale=1.0)

    pdwxd.release()
    pwork = tc.alloc_tile_pool(name="pfinal", bufs=2)
    ctx_pools.append(pwork)

    # -------- phase 6: attention conv tap-packed (broadcast reduce) --------
    do_attn = "attn" in phases
    A_pad = None
    if do_attn:
        A_pad = pwork.tile([41, PH, PW], F32R, name="A_pad", bufs=1)
        nc.gpsimd.memset(A_pad[:].bitcast(F32), 0.0)
        for nt in range(8):
            for ci in range(2):
                pA_t = psmall.tile([18, 512], F32, name=f"apsum{ci}",
                                   tag="small18")
                pA = pA_t[0:9, :]
                nc.tensor.matmul(
                    pA, saw2_sb[:, ci * 9:(ci + 1) * 9],
                    fused_pad[ci][:, nt * 8 + 1:nt * 8 + 9, 1:65],
                    start=True, stop=True)
                nc.scalar.activation(
                    A_pad[ci * 32:ci * 32 + 9, nt * 8 + 1:nt * 8 + 9, 1:65],
                    pA, ACT.Identity, scale=1.0)

    for nt in range(8):
        attn = pwork.tile([128, 512], F32, name="attn", tag="attn")
        if do_attn:
            ps = ppsum.tile([128, 512], F32, name="sapsum", tag="c1psum")
            first = True
            for t9 in range(9):
                dy, dx = t9 // 3 - 1, t9 % 3 - 1
                nc.tensor.matmul(
                    ps[:], asel_sb[:, t9 * 128:(t9 + 1) * 128],
                    A_pad[:, nt * 8 + 1 + dy:nt * 8 + 9 + dy, 1 + dx:65 + dx],
                    start=first, stop=(t9 == 8))
                first = False
            nc.scalar.activation(attn[:], ps[:], ACT.Sigmoid, bias=sab_sb[:, 0:1],
                                 scale=1.0)
        else:
            nc.gpsimd.memset(attn[:], 0.5)
        for co in range(2):
            xlt = pwork.tile([128, 512], F32, name="xlt", tag="xlt")
            nc.sync.dma_start(
                xlt[:], prm["xlp"][co * 128:(co + 1) * 128,
                                   nt * 8 + 1:(nt + 1) * 8 + 1, 1:65])
            ot = pwork.tile([128, 512], F32, name="ot", tag="ot")
            nc.vector.tensor_mul(
                ot[:], attn[:],
                diff_pad[co][:, nt * 8 + 1:nt * 8 + 9, 1:65].bitcast(F32))
            nc.vector.tensor_add(ot[:], ot[:], xlt[:])
            nc.sync.dma_start(
                prm["out"][co * 128:(co + 1) * 128, nt * 512:(nt + 1) * 512], ot[:])

    for p in reversed(ctx_pools):
        p.release()


def _build(repeat):
    nc = bacc.Bacc()
    prm = {}

    def din(name, shape, dt=F32):
        prm[name] = nc.declare_dram_parameter(name, list(shape), dt, isOutput=False)

    din("xlp", [C, PH, PW])
    din("xlph", [C, PH, PW], BF16)
    din("xlpl", [C, PH, PW], BF16)
    din("xhph", [C, PH, PW], BF16)
    din("xhpl", [C, PH, PW], BF16)
    din("xT2", [TBL_ROWS, 1024], BF16)
    din("w1Th", [128, 9216], BF16)
    din("w1Tl", [128, 9216], BF16)
    din("b1", [128, 2])
    din("w2ph", [128, 36], BF16)
    din("w2pl", [128, 36], BF16)
    din("sel", [18, 18], BF16)
    din("asel", [41, 1152], F32R)
    din("sawT2", [128, 18], F32R)
    din("ones1", [1, 128], F32R)
    din("bxy", [128, 64])
    din("ident", [128, 128])
    for k in range(3):
        din(f"dwT{k}", [128, 4608], F32R)
    din("db2", [128, 6])
    din("db_hw", [128, 6])
    din("se1T", [128, 384])
    din("se1b2", [64, 3])
    din("se2T", [64, 768])
    din("se2b2", [128, 6])
    din("sab_bc", [128, 1])
    prm["out"] = nc.declare_dram_parameter("out", [C, HW], F32, isOutput=True)

    with tile.TileContext(nc) as tc:
        if repeat == 1:
            _emit_body(nc, tc, prm)
        else:
            with tc.For_i(0, repeat, 1):
                _emit_body(nc, tc, prm)
    nc.finalize()
    return nc


def _prep_inputs(x_low, x_high, a1w, a1b, bn_g, bn_b, bn_m, bn_v, a2w, a2b,
                 dw, db, se1w, se1b, se2w, se2b, saw, sab):
    """Host-side weight prep shared by all cores + per-core activation prep."""
    import ml_dtypes
    f32 = np.float32
    bf16 = ml_dtypes.bfloat16
    # conv1 with BN folded
    scale = (bn_g / np.sqrt(bn_v + EPS)).astype(f32)  # [256]
    w1f = (a1w * scale[:, None, None, None]).astype(f32)  # [256,512,3,3]
    b1f = ((a1b - bn_m) * scale + bn_b).astype(f32)  # [256]
    # host lhsT layout [k(128), co(2), ty,tx, ci(4), m(128)] -> [128, 9216]
    arr = w1f.reshape(2, 128, 4, 128, 3, 3)  # [co, m, ci, k, ty, tx]
    w1T = np.ascontiguousarray(arr.transpose(3, 0, 4, 5, 2, 1)).reshape(128, 9216)
    w1Th = w1T.astype(bf16)
    w1Tl = (w1T - w1Th.astype(np.float32)).astype(bf16)
    b1h = np.ascontiguousarray(b1f.reshape(2, 128).T)  # [128, 2]

    # conv2 tap-packed, grid scale W/2 = 32 folded in
    # w2p[kk, cib*18 + c*9 + t] = 32*a2w[c, cib*128+kk, t]
    w2f = (a2w * 32.0).astype(f32).reshape(2, 2, 128, 9)  # [c, cib, kk, t]
    w2p = np.ascontiguousarray(w2f.transpose(2, 1, 0, 3)).reshape(128, 36)
    w2ph = w2p.astype(bf16)
    w2pl = (w2p - w2ph.astype(np.float32)).astype(bf16)
    # selection matrix for the shifted tap reduction: sel[r, t*2+c] = r==c*9+t
    sel = np.zeros((18, 18), f32)
    for c in range(2):
        for t in range(9):
            sel[c * 9 + t, t * 2 + c] = 1.0
    sel = sel.astype(bf16)

    # attention tap-packed: sawT2[kk, ci*9+t] = saw[0, ci*128+kk, t]
    sawT2 = np.ascontiguousarray(
        saw.astype(f32).reshape(2, 128, 9).transpose(1, 0, 2)).reshape(128, 18)
    # asel[r, t*128+m] = 1 if r % 9 == t (broadcast reduce to all partitions)
    asel = np.zeros((41, 9, 128), f32)
    for t in range(9):
        asel[t, t, :] = 1.0
        asel[32 + t, t, :] = 1.0
    asel = asel.reshape(41, 1152)

    # base grid (+a2b*32): pixel p = j*128+i ; h=p//64, w=p%64
    lin = np.linspace(-1.0, 1.0, 64, dtype=f32)
    pidx = (np.arange(32)[None, :] * 128 + np.arange(128)[:, None])  # [128,32]
    bx = ((lin[pidx // 64] + 1.0) * 32.0 - 0.5 + 32.0 * f32(a2b[0])).astype(f32)
    by = ((lin[pidx % 64] + 1.0) * 32.0 - 0.5 + 32.0 * f32(a2b[1])).astype(f32)
    bxy = np.empty((128, 64), f32)
    bxy[:, 0::2] = bx
    bxy[:, 1::2] = by

    # diff convs
    dwT = []
    for k in range(3):
        arr = dw[k].astype(f32).reshape(2, 128, 2, 128, 3, 3)  # [co,m,ci,kk,ty,tx]
        dwT.append(np.ascontiguousarray(arr.transpose(3, 4, 5, 2, 0, 1)).reshape(128, 4608))
    db2 = np.ascontiguousarray(db.astype(f32).reshape(3, 2, 128).transpose(2, 0, 1)
                               ).reshape(128, 6)
    db_hw = db2 * f32(HW)

    # SE (mean 1/HW folded into se1T)
    se1T = np.ascontiguousarray(
        (se1w.astype(f32) / HW).transpose(0, 2, 1).reshape(3, 2, 128, 64)
        .transpose(2, 0, 1, 3)).reshape(128, 384)
    se1b2 = np.ascontiguousarray(se1b.astype(f32).T)  # [64, 3]
    se2T = np.ascontiguousarray(
        se2w.astype(f32).transpose(0, 2, 1).reshape(3, 64, 2, 128)
        .transpose(1, 0, 2, 3)).reshape(64, 768)
    se2b2 = np.ascontiguousarray(se2b.astype(f32).reshape(3, 2, 128)
                                 .transpose(2, 0, 1)).reshape(128, 6)

    sab_bc = np.full((128, 1), f32(sab[0]), f32)

    shared = dict(w1Th=w1Th, w1Tl=w1Tl, b1=b1h, w2ph=w2ph, w2pl=w2pl, sel=sel,
                  asel=asel, sawT2=sawT2, ones1=np.ones((1, 128), f32),
                  bxy=bxy, ident=np.eye(128, dtype=f32),
                  dwT0=dwT[0], dwT1=dwT[1], dwT2=dwT[2], db2=db2, db_hw=db_hw,
                  se1T=se1T, se1b2=se1b2, se2T=se2T, se2b2=se2b2,
                  sab_bc=sab_bc)

    in_maps = []
    for b in range(B):
        xlb = np.ascontiguousarray(x_low[b].astype(f32))
        xhb = np.ascontiguousarray(x_high[b].astype(f32))
        xlp = np.zeros((C, PH, PW), f32)
        xlp[:, 1:1 + H, 1:1 + W] = xlb
        xhp = np.zeros((C, PH, PW), f32)
        xhp[:, 1:1 + H, 1:1 + W] = xhb
        xlph = xlp.astype(bf16)
        xlpl = (xlp - xlph.astype(f32)).astype(bf16)
        xhph = xhp.astype(bf16)
        xhpl = (xhp - xhph.astype(f32)).astype(bf16)
        # patch table: row (py*66+px) = [v00|v01|v10|v11] of padded x_high,
        # guard ring of zeros handles clamped out-of-image taps (weights are 0)
        xh67 = np.zeros((C, 67, 67), bf16)
        xh67[:, 1:1 + H, 1:1 + W] = xhb.astype(bf16)
        sw = np.lib.stride_tricks.sliding_window_view(
            xh67, (2, 2), axis=(1, 2))  # [C, 66, 66, 2, 2]
        XT2 = np.zeros((TBL_ROWS, 1024), bf16)
        XT2[:66 * 66] = np.ascontiguousarray(
            sw.transpose(1, 2, 3, 4, 0)).reshape(66 * 66, 1024)
        m = dict(shared)
        m["xlp"] = xlp
        m["xlph"] = xlph
        m["xlpl"] = xlpl
        m["xhph"] = xhph
        m["xhpl"] = xhpl
        m["xT2"] = XT2
        in_maps.append(m)
    return in_maps


_last_results = None


def kernel(**inputs):
    global _last_results
    repeat = int(os.environ.get("KERNEL_REPEAT", "1"))
    if repeat not in _nc_cache:
        _nc_cache[repeat] = _build(repeat)
    nc = _nc_cache[repeat]
    in_maps = _prep_inputs(**inputs)
    res = run_bass_kernel_spmd(nc, in_maps, list(range(NCORES)))
    _last_results = res
    out = np.stack([res.results[b]["out"].reshape(C, H, W) for b in range(B)])
    return out.astype(np.float32)


if __name__ == "__main__":
    import reference
    inputs = {k: np.asarray(v) for k, v in reference.setup_inputs().items()}
    expected = np.asarray(reference.reference(**inputs))
    actual = kernel(**inputs)
    err = np.abs(actual - expected).max()
    rel = err / np.abs(expected).max()
    print(f"abs err: {err:.4e}  rel err: {rel:.4e}")
